# revision 6
# baseline (speedup 1.0000x reference)
"""Trainium2 Bass kernel for nn_NeuralODEFunc (3-layer gated GCN) on 8 NeuronCores.

Strategy (self-contained, hardcoded for N=50000, D=512, E=160000, 8 cores):
  - Nodes sharded across 8 cores (6250/core, padded to 6272 = 49 blocks of 128).
  - Edges partitioned by destination core/block; scatter-add is expressed as
    PE matmuls with per-block one-hot "S" matrices (norm baked in), built on host.
  - Per GCN layer: local XW matmul (bf16) -> AllGather (bf16) -> indirect-DMA
    gather of source rows -> S-matmul accumulate in PSUM -> bias + LayerNorm
    (bn_stats + ACT affine) -> PE transpose back to feature-major state ->
    gated blend (layers 1,2) via transposed gate matmuls + sigmoid.
  - Final tanh + residual_weight * h_orig, emitted row-major fp32.

State lives in SBUF as hT [4][128, 6272] bf16 (feature-major), double buffered.
"""

import time

import numpy as np
import ml_dtypes

import concourse.bass as bass
import concourse.mybir as mybir
import concourse.tile as tile
from concourse.bass_utils import run_bass_kernel_spmd
from concourse.masks import make_identity

NC = 8
P = 128
D = 512
N = 50000
E = 160000
SHARD = N // NC          # 6250
NB = (SHARD + P - 1) // P  # 49
SH = NB * P              # 6272
NPAD = NC * SH           # 50176
KC = D // P              # 4
LAYERS = 3
LN_EPS = 1e-5

bf16 = mybir.dt.bfloat16
f32 = mybir.dt.float32
i32 = mybir.dt.int32
AF = mybir.ActivationFunctionType
ALU = mybir.AluOpType


# ---------------------------------------------------------------- wait split
def _split_excess_waits(nc, max_waits=1):
    """This walrus build supports only ONE embedded sync wait per instruction.
    Move excess waits onto preceding same-engine NOPs (one wait each)."""
    n_split = 0
    for bb in nc.main_func.blocks:
        out = []
        changed = False
        for ins in bb.instructions:
            si = ins.sync_info
            if si is not None and len(si.on_wait) > max_waits:
                waits = list(si.on_wait)
                excess = waits[:-max_waits]
                keep = waits[-max_waits:]
                for w in excess:
                    nop = mybir.InstNoOp(
                        name=nc.get_next_instruction_name(),
                        text_hint="waitsplit",
                        bass_nofuse=True,
                    )
                    nop.engine = ins.engine
                    nop.sync_info = mybir.SyncInfo(on_wait=[w], on_update=[])
                    nc.register_instruction(nop)
                    out.append(nop)
                    n_split += 1
                ins.sync_info = mybir.SyncInfo(
                    on_wait=keep, on_update=list(si.on_update)
                )
                changed = True
            out.append(ins)
        if changed:
            bb.instructions = out
    return n_split


# ---------------------------------------------------------------- host prep
def _host_prep(h, edge_index, W_gcn, b_gcn, ln_gamma, ln_beta, W_gate, b_gate,
               residual_weight):
    src = np.asarray(edge_index[0], dtype=np.int64)
    dst = np.asarray(edge_index[1], dtype=np.int64)

    deg = np.bincount(dst, minlength=N).astype(np.float32)
    dinv = np.where(deg > 0, 1.0 / np.sqrt(np.maximum(deg, 1.0)), 0.0).astype(
        np.float32
    )
    norm = (dinv[src] * dinv[dst]).astype(np.float32)

    core_of = (dst // SHARD).astype(np.int64)
    loc = dst % SHARD
    blk = loc // P
    dloc = (loc % P).astype(np.int64)
    gsrc = ((src // SHARD) * SH + (src % SHARD)).astype(np.int32)

    # per (core, block) edge counts -> shared per-block chunk counts
    counts = np.zeros((NC, NB), np.int64)
    np.add.at(counts, (core_of, blk), 1)
    Kb = np.maximum(1, -(-counts.max(axis=0) // P)).astype(np.int64)  # ceil
    qofs = np.concatenate([[0], np.cumsum(Kb)]).astype(np.int64)
    Ksum = int(qofs[-1])

    # stable sort edges by (core, blk); position within group -> (chunk, lane)
    key = core_of * NB + blk
    order = np.argsort(key, kind="stable")
    skey = key[order]
    grp_start_mask = np.ones(E, dtype=bool)
    grp_start_mask[1:] = skey[1:] != skey[:-1]
    grp_start_idx = np.flatnonzero(grp_start_mask)
    grp_id = np.cumsum(grp_start_mask) - 1
    pos = np.arange(E) - grp_start_idx[grp_id]

    s_core = core_of[order]
    s_blk = blk[order]
    s_dloc = dloc[order]
    s_norm = norm[order]
    s_gsrc = gsrc[order]
    chunk = qofs[s_blk] + pos // P
    lane = pos % P

    S = np.zeros((NC, Ksum, P, P), np.float32)
    S[s_core, chunk, lane, s_dloc] = s_norm
    S = S.astype(ml_dtypes.bfloat16)
    idx = np.zeros((NC, Ksum, P), np.int32)
    idx[s_core, chunk, lane] = s_gsrc

    # padded node shards + transposed bf16 initial state
    h = np.asarray(h, dtype=np.float32)
    h_pad = np.zeros((NC, SH, D), np.float32)
    h_pad[:, :SHARD, :] = h.reshape(NC, SHARD, D)
    hT0 = (
        h_pad.transpose(0, 2, 1)  # [NC, D, SH]
        .reshape(NC, KC, P, SH)
        .astype(ml_dtypes.bfloat16)
    )

    Wg = np.asarray(W_gcn, np.float32).reshape(LAYERS, KC, P, D).astype(
        ml_dtypes.bfloat16
    )
    Wgate = np.asarray(W_gate, np.float32).reshape(2 * KC, P, KC, P).astype(
        ml_dtypes.bfloat16
    )
    gam = np.asarray(ln_gamma, np.float32).reshape(LAYERS, KC, P)
    bet = np.asarray(ln_beta, np.float32).reshape(LAYERS, KC, P)
    bgt = np.asarray(b_gate, np.float32).reshape(KC, P)
    bgc = np.asarray(b_gcn, np.float32)
    rw = np.asarray(residual_weight, np.float32).reshape(1, 1)

    in_maps = []
    for c in range(NC):
        in_maps.append(
            {
                "hT0": hT0[c],
                "h0": h_pad[c],
                "S": S[c],
                "idx": idx[c],
                "Wgcn": Wg,
                "Wgate": Wgate,
                "bgcn": bgc,
                "gam": gam,
                "bet": bet,
                "bgate": bgt,
                "rw": rw,
            }
        )
    return in_maps, tuple(int(k) for k in Kb)


# ---------------------------------------------------------------- device build
def _build(Kb):
    qofs = [0]
    for k in Kb:
        qofs.append(qofs[-1] + k)
    Ksum = qofs[-1]

    nc = bass.Bass()

    hT0_d = nc.dram_tensor("hT0", [KC, P, SH], bf16, kind="ExternalInput")
    h0_d = nc.dram_tensor("h0", [SH, D], f32, kind="ExternalInput")
    S_d = nc.dram_tensor("S", [Ksum, P, P], bf16, kind="ExternalInput")
    idx_d = nc.dram_tensor("idx", [Ksum, P], i32, kind="ExternalInput")
    Wgcn_d = nc.dram_tensor("Wgcn", [LAYERS, KC, P, D], bf16, kind="ExternalInput")
    Wgate_d = nc.dram_tensor("Wgate", [2 * KC, P, KC, P], bf16, kind="ExternalInput")
    bgcn_d = nc.dram_tensor("bgcn", [LAYERS, D], f32, kind="ExternalInput")
    gam_d = nc.dram_tensor("gam", [LAYERS, KC, P], f32, kind="ExternalInput")
    bet_d = nc.dram_tensor("bet", [LAYERS, KC, P], f32, kind="ExternalInput")
    bgate_d = nc.dram_tensor("bgate", [KC, P], f32, kind="ExternalInput")
    rw_d = nc.dram_tensor("rw", [1, 1], f32, kind="ExternalInput")
    out_d = nc.dram_tensor("out", [SH, D], f32, kind="ExternalOutput")

    ag_in = [nc.dram_tensor(f"ag_in{l}", [SH, D], bf16) for l in range(LAYERS)]
    ag_out = [
        nc.dram_tensor(f"ag_out{l}", [NPAD, D], bf16, addr_space="Shared")
        for l in range(LAYERS)
    ]

    # free tiles over SH for the gate phase
    ftiles = []
    o = 0
    while o < SH:
        w = min(512, SH - o)
        ftiles.append((o, w))
        o += w

    with tile.TileContext(nc) as tc:
        with (
            tc.tile_pool(name="const", bufs=1) as const,
            tc.tile_pool(name="state", bufs=1) as state,
            tc.tile_pool(name="xwp", bufs=3) as xwp,
            tc.tile_pool(name="msg", bufs=6) as msg,
            tc.tile_pool(name="spool", bufs=6) as spool,
            tc.tile_pool(name="xsp", bufs=3) as xsp,
            tc.tile_pool(name="stat", bufs=12) as stat,
            tc.tile_pool(name="corep", bufs=3) as corep,
            tc.tile_pool(name="gtile", bufs=3) as gtile,
            tc.tile_pool(name="dtmp", bufs=3) as dtmp,
            tc.tile_pool(name="finp", bufs=3) as finp,
            tc.tile_pool(name="pxw", bufs=2, space="PSUM") as pxw,
            tc.tile_pool(name="pagg", bufs=2, space="PSUM") as pagg,
            tc.tile_pool(name="ptr", bufs=2, space="PSUM") as ptr,
            tc.tile_pool(name="pg", bufs=2, space="PSUM") as pg,
        ):
            ident = const.tile([P, P], bf16)
            make_identity(nc, ident)
            eps_t = const.tile([P, 1], f32)
            nc.vector.memset(eps_t[:], LN_EPS)

            W_sb = const.tile([P, LAYERS, KC, D], bf16)
            nc.sync.dma_start(
                out=W_sb[:], in_=Wgcn_d.rearrange("l k p d -> p l k d")
            )
            Wg_sb = const.tile([P, 2 * KC, KC, P], bf16)
            nc.sync.dma_start(
                out=Wg_sb[:], in_=Wgate_d.rearrange("k p f c -> p k f c")
            )
            b_sb = const.tile([P, LAYERS, D], f32)
            nc.sync.dma_start(
                out=b_sb[:],
                in_=bass.AP(
                    tensor=bgcn_d[:].tensor,
                    offset=0,
                    ap=[[0, P]] + list(bgcn_d[:].ap),
                ),
            )
            gam_sb = const.tile([P, LAYERS, KC], f32)
            nc.sync.dma_start(out=gam_sb[:], in_=gam_d.rearrange("l k p -> p l k"))
            bet_sb = const.tile([P, LAYERS, KC], f32)
            nc.sync.dma_start(out=bet_sb[:], in_=bet_d.rearrange("l k p -> p l k"))
            bg_sb = const.tile([P, KC], f32)
            nc.sync.dma_start(out=bg_sb[:], in_=bgate_d.rearrange("f p -> p f"))
            rw_sb = const.tile([P, 1], f32)
            nc.sync.dma_start(
                out=rw_sb[:],
                in_=bass.AP(tensor=rw_d[:].tensor, offset=0, ap=[[0, P], [1, 1]]),
            )
            idx_sb = const.tile([P, Ksum], i32)
            nc.sync.dma_start(out=idx_sb[:], in_=idx_d.rearrange("q p -> p q"))
            probe = const.tile([P, 1], bf16)

            hT = [state.tile([P, SH], bf16, tag=f"hA{k}", name=f"hA{k}") for k in range(KC)]
            hN = [state.tile([P, SH], bf16, tag=f"hB{k}", name=f"hB{k}") for k in range(KC)]
            for k in range(KC):
                nc.sync.dma_start(out=hT[k][:], in_=hT0_d[k])

            for l in range(LAYERS):
                # ---- phase A: XW = h @ W_l  (row-major bf16) -> ag_in
                for b in range(NB):
                    ps = pxw.tile([P, D], f32)
                    for k in range(KC):
                        nc.tensor.matmul(
                            ps[:],
                            lhsT=hT[k][:, b * P : (b + 1) * P],
                            rhs=W_sb[:, l, k, :],
                            start=(k == 0),
                            stop=(k == KC - 1),
                        )
                    xw = xwp.tile([P, D], bf16)
                    nc.scalar.activation(out=xw[:], in_=ps[:], func=AF.Copy)
                    nc.sync.dma_start(
                        out=ag_in[l][b * P : (b + 1) * P, :], in_=xw[:]
                    )

                # ---- AllGather full XW (bf16)
                nc.gpsimd.collective_compute(
                    "AllGather",
                    ALU.bypass,
                    ins=[ag_in[l][:]],
                    outs=[ag_out[l][:]],
                    replica_groups=[list(range(NC))],
                )
                # collapse the collective dep into the POOL queue
                nc.gpsimd.dma_start(out=probe[:1, :1], in_=ag_out[l][0:1, 0:1])

                # ---- phase B+C: gather, scatter-matmul, bias+LN, transpose
                for b in range(NB):
                    ps = pagg.tile([P, D], f32)
                    for j in range(Kb[b]):
                        q = qofs[b] + j
                        m = msg.tile([P, D], bf16)
                        nc.gpsimd.indirect_dma_start(
                            out=m[:],
                            out_offset=None,
                            in_=ag_out[l][:],
                            in_offset=bass.IndirectOffsetOnAxis(
                                ap=idx_sb[:, q : q + 1], axis=0
                            ),
                        )
                        s_t = spool.tile([P, P], bf16, tag="s_t")
                        nc.sync.dma_start(out=s_t[:], in_=S_d[q])
                        nc.tensor.matmul(
                            ps[:],
                            lhsT=s_t[:],
                            rhs=m[:],
                            start=(j == 0),
                            stop=(j == Kb[b] - 1),
                        )
                    xs = xsp.tile([P, D], f32)
                    nc.vector.tensor_tensor(
                        out=xs[:], in0=ps[:], in1=b_sb[:, l, :], op=ALU.add
                    )
                    st = stat.tile([P, 6], f32)
                    nc.vector.bn_stats(out=st[:], in_=xs[:])
                    mv = stat.tile([P, 2], f32)
                    nc.vector.bn_aggr(out=mv[:], in_=st[:])
                    sd = stat.tile([P, 1], f32)
                    nc.scalar.activation(
                        out=sd[:], in_=mv[:, 1:2], func=AF.Sqrt, bias=eps_t[:, :1]
                    )
                    rstd = stat.tile([P, 1], f32)
                    nc.vector.reciprocal(out=rstd[:], in_=sd[:])
                    nmu = stat.tile([P, 1], f32)
                    nc.vector.tensor_tensor(
                        out=nmu[:], in0=mv[:, 0:1], in1=rstd[:], op=ALU.mult
                    )
                    nc.vector.tensor_scalar_mul(
                        out=nmu[:], in0=nmu[:], scalar1=-1.0
                    )
                    core = corep.tile([P, D], bf16)
                    nc.scalar.activation(
                        out=core[:],
                        in_=xs[:],
                        func=AF.Identity,
                        scale=rstd[:, :1],
                        bias=nmu[:, :1],
                    )
                    for k in range(KC):
                        tp = ptr.tile([P, P], bf16)
                        nc.tensor.transpose(
                            out=tp[:],
                            in_=core[:, k * P : (k + 1) * P],
                            identity=ident[:],
                        )
                        nc.scalar.activation(
                            out=hN[k][:, b * P : (b + 1) * P],
                            in_=tp[:],
                            func=AF.Identity,
                            scale=gam_sb[:, l, k : k + 1],
                            bias=bet_sb[:, l, k : k + 1],
                        )

                # ---- phase D: gate + blend (layers 1, 2)
                if l > 0:
                    for (t0, tw) in ftiles:
                        # compute ALL gate chunks for this tile before any
                        # blend writes hN (the matmuls read hN as input)
                        gs = []
                        for f in range(KC):
                            pg_ = pg.tile([P, 512], f32, tag="pg")
                            for k in range(2 * KC):
                                rhs_t = (hT if k < KC else hN)[k % KC]
                                nc.tensor.matmul(
                                    pg_[:, :tw],
                                    lhsT=Wg_sb[:, k, f, :],
                                    rhs=rhs_t[:, t0 : t0 + tw],
                                    start=(k == 0),
                                    stop=(k == 2 * KC - 1),
                                )
                            g = gtile.tile(
                                [P, 512], bf16, tag=f"g{f}", name=f"g{f}"
                            )
                            nc.scalar.activation(
                                out=g[:, :tw],
                                in_=pg_[:, :tw],
                                func=AF.Sigmoid,
                                bias=bg_sb[:, f : f + 1],
                            )
                            gs.append(g)
                        for f in range(KC):
                            d_ = dtmp.tile(
                                [P, 512], bf16, tag=f"d{f}", name=f"d{f}"
                            )
                            nc.vector.tensor_tensor(
                                out=d_[:, :tw],
                                in0=hN[f][:, t0 : t0 + tw],
                                in1=hT[f][:, t0 : t0 + tw],
                                op=ALU.subtract,
                            )
                            nc.vector.tensor_tensor(
                                out=d_[:, :tw],
                                in0=gs[f][:, :tw],
                                in1=d_[:, :tw],
                                op=ALU.mult,
                            )
                            nc.vector.tensor_tensor(
                                out=hN[f][:, t0 : t0 + tw],
                                in0=hT[f][:, t0 : t0 + tw],
                                in1=d_[:, :tw],
                                op=ALU.add,
                            )
                hT, hN = hN, hT

            # ---- final: out = tanh(h) + rw * h_orig
            for b in range(NB):
                ob = finp.tile([P, D], f32, tag="ob")
                for k in range(KC):
                    tp = ptr.tile([P, P], bf16)
                    nc.tensor.transpose(
                        out=tp[:],
                        in_=hT[k][:, b * P : (b + 1) * P],
                        identity=ident[:],
                    )
                    nc.scalar.activation(
                        out=ob[:, k * P : (k + 1) * P], in_=tp[:], func=AF.Tanh
                    )
                h0b = finp.tile([P, D], f32, tag="h0b")
                nc.sync.dma_start(out=h0b[:], in_=h0_d[b * P : (b + 1) * P, :])
                rt = finp.tile([P, D], f32, tag="rt")
                nc.vector.tensor_scalar_mul(
                    out=rt[:], in0=h0b[:], scalar1=rw_sb[:, :1]
                )
                nc.vector.tensor_tensor(
                    out=ob[:], in0=ob[:], in1=rt[:], op=ALU.add
                )
                nc.sync.dma_start(out=out_d[b * P : (b + 1) * P, :], in_=ob[:])

    _split_excess_waits(nc)
    return nc


_BUILD_CACHE = {}


def _get_nc(Kb):
    if Kb not in _BUILD_CACHE:
        _BUILD_CACHE[Kb] = _build(Kb)
    return _BUILD_CACHE[Kb]


def kernel(t=None, h=None, edge_index=None, W_gcn=None, b_gcn=None,
           ln_gamma=None, ln_beta=None, W_gate=None, b_gate=None,
           residual_weight=None, **_ignored):
    in_maps, Kb = _host_prep(
        h, edge_index, W_gcn, b_gcn, ln_gamma, ln_beta, W_gate, b_gate,
        residual_weight,
    )
    nc = _get_nc(Kb)
    res = run_bass_kernel_spmd(nc, in_maps, list(range(NC)))
    outs = [res.results[c]["out"][:SHARD] for c in range(NC)]
    return np.concatenate(outs, axis=0).astype(np.float32)


# revision 10
# speedup vs baseline: 33.3515x; 33.3515x over previous
"""Trainium2 Bass kernel for nn_NeuralODEFunc (3-layer gated GCN) on 8 NeuronCores.

Strategy (self-contained, hardcoded for N=50000, D=512, E=160000, 8 cores):
  - Nodes sharded across 8 cores (6250/core, padded to 6272 = 49 blocks of 128).
  - Edges partitioned by destination core/block; scatter-add is expressed as
    PE matmuls with per-block one-hot "S" matrices (norm baked in), built on host.
  - Per GCN layer: local XW matmul (bf16) -> AllGather (bf16) -> indirect-DMA
    gather of source rows -> S-matmul accumulate in PSUM -> bias + LayerNorm
    (bn_stats + ACT affine) -> PE transpose back to feature-major state ->
    gated blend (layers 1,2) via transposed gate matmuls + sigmoid.
  - Final tanh + residual_weight * h_orig, emitted row-major fp32.

State lives in SBUF as hT [4][128, 6272] bf16 (feature-major), double buffered.
"""

import time

import numpy as np
import ml_dtypes

import concourse.bass as bass
import concourse.mybir as mybir
import concourse.tile as tile
from concourse.bass_utils import run_bass_kernel_spmd
from concourse.masks import make_identity

NC = 8
P = 128
D = 512
N = 50000
E = 160000
SHARD = N // NC          # 6250
NB = (SHARD + P - 1) // P  # 49
SH = NB * P              # 6272
NPAD = NC * SH           # 50176
KC = D // P              # 4
LAYERS = 3
LN_EPS = 1e-5

bf16 = mybir.dt.bfloat16
f32 = mybir.dt.float32
i32 = mybir.dt.int32
AF = mybir.ActivationFunctionType
ALU = mybir.AluOpType


# ---------------------------------------------------------------- wait split
def _split_excess_waits(nc, max_waits=1):
    """This walrus build supports only ONE embedded sync wait per instruction.
    Move excess waits onto preceding same-engine NOPs (one wait each)."""
    n_split = 0
    for bb in nc.main_func.blocks:
        out = []
        changed = False
        for ins in bb.instructions:
            si = ins.sync_info
            if si is not None and len(si.on_wait) > max_waits:
                waits = list(si.on_wait)
                excess = waits[:-max_waits]
                keep = waits[-max_waits:]
                for w in excess:
                    nop = mybir.InstNoOp(
                        name=nc.get_next_instruction_name(),
                        text_hint="waitsplit",
                        bass_nofuse=True,
                    )
                    nop.engine = ins.engine
                    nop.sync_info = mybir.SyncInfo(on_wait=[w], on_update=[])
                    nc.register_instruction(nop)
                    out.append(nop)
                    n_split += 1
                ins.sync_info = mybir.SyncInfo(
                    on_wait=keep, on_update=list(si.on_update)
                )
                changed = True
            out.append(ins)
        if changed:
            bb.instructions = out
    return n_split


# ---------------------------------------------------------------- host prep
def _host_prep(h, edge_index, W_gcn, b_gcn, ln_gamma, ln_beta, W_gate, b_gate,
               residual_weight):
    src = np.asarray(edge_index[0], dtype=np.int64)
    dst = np.asarray(edge_index[1], dtype=np.int64)

    deg = np.bincount(dst, minlength=N).astype(np.float32)
    dinv = np.where(deg > 0, 1.0 / np.sqrt(np.maximum(deg, 1.0)), 0.0).astype(
        np.float32
    )
    norm = (dinv[src] * dinv[dst]).astype(np.float32)

    core_of = (dst // SHARD).astype(np.int64)
    loc = dst % SHARD
    blk = loc // P
    dloc = (loc % P).astype(np.int64)
    gsrc = ((src // SHARD) * SH + (src % SHARD)).astype(np.int32)

    # per (core, block) edge counts -> shared per-block chunk counts
    counts = np.zeros((NC, NB), np.int64)
    np.add.at(counts, (core_of, blk), 1)
    Kb = np.maximum(1, -(-counts.max(axis=0) // P)).astype(np.int64)  # ceil
    qofs = np.concatenate([[0], np.cumsum(Kb)]).astype(np.int64)
    Ksum = int(qofs[-1])

    # stable sort edges by (core, blk); position within group -> (chunk, lane)
    key = core_of * NB + blk
    order = np.argsort(key, kind="stable")
    skey = key[order]
    grp_start_mask = np.ones(E, dtype=bool)
    grp_start_mask[1:] = skey[1:] != skey[:-1]
    grp_start_idx = np.flatnonzero(grp_start_mask)
    grp_id = np.cumsum(grp_start_mask) - 1
    pos = np.arange(E) - grp_start_idx[grp_id]

    s_core = core_of[order]
    s_blk = blk[order]
    s_dloc = dloc[order]
    s_norm = norm[order]
    s_gsrc = gsrc[order]
    chunk = qofs[s_blk] + pos // P
    lane = pos % P

    S = np.zeros((NC, Ksum, P, P), np.float32)
    S[s_core, chunk, lane, s_dloc] = s_norm
    S = S.astype(ml_dtypes.bfloat16)
    idx = np.zeros((NC, Ksum, P), np.int32)
    idx[s_core, chunk, lane] = s_gsrc

    # padded node shards + transposed bf16 initial state
    h = np.asarray(h, dtype=np.float32)
    h_pad = np.zeros((NC, SH, D), np.float32)
    h_pad[:, :SHARD, :] = h.reshape(NC, SHARD, D)
    hT0 = (
        h_pad.transpose(0, 2, 1)  # [NC, D, SH]
        .reshape(NC, KC, P, SH)
        .astype(ml_dtypes.bfloat16)
    )

    Wg = np.asarray(W_gcn, np.float32).reshape(LAYERS, KC, P, D).astype(
        ml_dtypes.bfloat16
    )
    Wgate = np.asarray(W_gate, np.float32).reshape(2 * KC, P, KC, P).astype(
        ml_dtypes.bfloat16
    )
    gam = np.asarray(ln_gamma, np.float32).reshape(LAYERS, KC, P)
    bet = np.asarray(ln_beta, np.float32).reshape(LAYERS, KC, P)
    bgt = np.asarray(b_gate, np.float32).reshape(KC, P)
    bgc = np.asarray(b_gcn, np.float32)
    rw = np.asarray(residual_weight, np.float32).reshape(1, 1)

    in_maps = []
    for c in range(NC):
        in_maps.append(
            {
                "hT0": hT0[c],
                "h0": h_pad[c],
                "S": S[c],
                "idx": idx[c],
                "Wgcn": Wg,
                "Wgate": Wgate,
                "bgcn": bgc,
                "gam": gam,
                "bet": bet,
                "bgate": bgt,
                "rw": rw,
            }
        )
    return in_maps, tuple(int(k) for k in Kb)


# ---------------------------------------------------------------- device build
def _build(Kb, repeats=1):
    qofs = [0]
    for k in Kb:
        qofs.append(qofs[-1] + k)
    Ksum = qofs[-1]

    nc = bass.Bass()

    hT0_d = nc.dram_tensor("hT0", [KC, P, SH], bf16, kind="ExternalInput")
    h0_d = nc.dram_tensor("h0", [SH, D], f32, kind="ExternalInput")
    S_d = nc.dram_tensor("S", [Ksum, P, P], bf16, kind="ExternalInput")
    idx_d = nc.dram_tensor("idx", [Ksum, P], i32, kind="ExternalInput")
    Wgcn_d = nc.dram_tensor("Wgcn", [LAYERS, KC, P, D], bf16, kind="ExternalInput")
    Wgate_d = nc.dram_tensor("Wgate", [2 * KC, P, KC, P], bf16, kind="ExternalInput")
    bgcn_d = nc.dram_tensor("bgcn", [LAYERS, D], f32, kind="ExternalInput")
    gam_d = nc.dram_tensor("gam", [LAYERS, KC, P], f32, kind="ExternalInput")
    bet_d = nc.dram_tensor("bet", [LAYERS, KC, P], f32, kind="ExternalInput")
    bgate_d = nc.dram_tensor("bgate", [KC, P], f32, kind="ExternalInput")
    rw_d = nc.dram_tensor("rw", [1, 1], f32, kind="ExternalInput")
    out_d = nc.dram_tensor("out", [SH, D], f32, kind="ExternalOutput")

    ag_in = [
        nc.dram_tensor(f"ag_in{l}", [SH, D], bf16)
        for l in range(LAYERS * repeats)
    ]
    ag_out = [
        nc.dram_tensor(f"ag_out{l}", [NPAD, D], bf16, addr_space="Shared")
        for l in range(LAYERS * repeats)
    ]

    # free tiles over SH for the gate phase
    ftiles = []
    o = 0
    while o < SH:
        w = min(512, SH - o)
        ftiles.append((o, w))
        o += w

    with tile.TileContext(nc) as tc:
        with (
            tc.tile_pool(name="const", bufs=1) as const,
            tc.tile_pool(name="state", bufs=1) as state,
            tc.tile_pool(name="xwp", bufs=3) as xwp,
            tc.tile_pool(name="msg", bufs=6) as msg,
            tc.tile_pool(name="spool", bufs=6) as spool,
            tc.tile_pool(name="xsp", bufs=3) as xsp,
            tc.tile_pool(name="stat", bufs=12) as stat,
            tc.tile_pool(name="corep", bufs=3) as corep,
            tc.tile_pool(name="gtile", bufs=3) as gtile,
            tc.tile_pool(name="dtmp", bufs=3) as dtmp,
            tc.tile_pool(name="finp", bufs=3) as finp,
            tc.tile_pool(name="pxw", bufs=2, space="PSUM") as pxw,
            tc.tile_pool(name="pagg", bufs=2, space="PSUM") as pagg,
            tc.tile_pool(name="ptr", bufs=2, space="PSUM") as ptr,
            tc.tile_pool(name="pg", bufs=2, space="PSUM") as pg,
        ):
            ident = const.tile([P, P], bf16)
            make_identity(nc, ident)
            eps_t = const.tile([P, 1], f32)
            nc.vector.memset(eps_t[:], LN_EPS)

            W_sb = const.tile([P, LAYERS, KC, D], bf16)
            nc.sync.dma_start(
                out=W_sb[:], in_=Wgcn_d.rearrange("l k p d -> p l k d")
            )
            Wg_sb = const.tile([P, 2 * KC, KC, P], bf16)
            nc.sync.dma_start(
                out=Wg_sb[:], in_=Wgate_d.rearrange("k p f c -> p k f c")
            )
            b_sb = const.tile([P, LAYERS, D], f32)
            nc.sync.dma_start(
                out=b_sb[:],
                in_=bass.AP(
                    tensor=bgcn_d[:].tensor,
                    offset=0,
                    ap=[[0, P]] + list(bgcn_d[:].ap),
                ),
            )
            gam_sb = const.tile([P, LAYERS, KC], f32)
            nc.sync.dma_start(out=gam_sb[:], in_=gam_d.rearrange("l k p -> p l k"))
            bet_sb = const.tile([P, LAYERS, KC], f32)
            nc.sync.dma_start(out=bet_sb[:], in_=bet_d.rearrange("l k p -> p l k"))
            bg_sb = const.tile([P, KC], f32)
            nc.sync.dma_start(out=bg_sb[:], in_=bgate_d.rearrange("f p -> p f"))
            rw_sb = const.tile([P, 1], f32)
            nc.sync.dma_start(
                out=rw_sb[:],
                in_=bass.AP(tensor=rw_d[:].tensor, offset=0, ap=[[0, P], [1, 1]]),
            )
            idx_sb = const.tile([P, Ksum], i32)
            nc.sync.dma_start(out=idx_sb[:], in_=idx_d.rearrange("q p -> p q"))
            probe = const.tile([P, 1], bf16)

            hT = [state.tile([P, SH], bf16, tag=f"hA{k}", name=f"hA{k}") for k in range(KC)]
            hN = [state.tile([P, SH], bf16, tag=f"hB{k}", name=f"hB{k}") for k in range(KC)]

            # repeats > 1 is only for device-time measurement (slope method)
            for rep in range(repeats):
                for k in range(KC):
                    nc.sync.dma_start(out=hT[k][:], in_=hT0_d[k])

                for ll in range(LAYERS):
                    lg = rep * LAYERS + ll  # ag buffer index
                    # ---- phase A: XW = h @ W_l (row-major bf16) -> ag_in
                    for b in range(NB):
                        ps = pxw.tile([P, D], f32, tag="pxw", name="pxw")
                        for k in range(KC):
                            nc.tensor.matmul(
                                ps[:],
                                lhsT=hT[k][:, b * P : (b + 1) * P],
                                rhs=W_sb[:, ll, k, :],
                                start=(k == 0),
                                stop=(k == KC - 1),
                            )
                        xw = xwp.tile([P, D], bf16, tag="xw", name="xw")
                        nc.scalar.activation(out=xw[:], in_=ps[:], func=AF.Copy)
                        nc.sync.dma_start(
                            out=ag_in[lg][b * P : (b + 1) * P, :], in_=xw[:]
                        )

                    # ---- AllGather full XW (bf16)
                    nc.gpsimd.collective_compute(
                        "AllGather",
                        ALU.bypass,
                        ins=[ag_in[lg][:]],
                        outs=[ag_out[lg][:]],
                        replica_groups=[list(range(NC))],
                    )
                    # collapse the collective dep into the POOL queue
                    nc.gpsimd.dma_start(
                        out=probe[:1, :1], in_=ag_out[lg][0:1, 0:1]
                    )

                    # ---- phase B+C: gather, scatter-matmul, bias+LN, transpose
                    for b in range(NB):
                        ps = pagg.tile([P, D], f32, tag="pagg", name="pagg")
                        for j in range(Kb[b]):
                            q = qofs[b] + j
                            m = msg.tile([P, D], bf16, tag="m", name="m")
                            nc.gpsimd.indirect_dma_start(
                                out=m[:],
                                out_offset=None,
                                in_=ag_out[lg][:],
                                in_offset=bass.IndirectOffsetOnAxis(
                                    ap=idx_sb[:, q : q + 1], axis=0
                                ),
                            )
                            s_t = spool.tile([P, P], bf16, tag="s_t", name="s_t")
                            nc.sync.dma_start(out=s_t[:], in_=S_d[q])
                            nc.tensor.matmul(
                                ps[:],
                                lhsT=s_t[:],
                                rhs=m[:],
                                start=(j == 0),
                                stop=(j == Kb[b] - 1),
                            )
                        xs = xsp.tile([P, D], f32, tag="xs", name="xs")
                        nc.vector.tensor_tensor(
                            out=xs[:], in0=ps[:], in1=b_sb[:, ll, :], op=ALU.add
                        )
                        st = stat.tile([P, 6], f32, tag="st", name="st")
                        nc.vector.bn_stats(out=st[:], in_=xs[:])
                        mv = stat.tile([P, 2], f32, tag="mv", name="mv")
                        nc.vector.bn_aggr(out=mv[:], in_=st[:])
                        sd = stat.tile([P, 1], f32, tag="sd", name="sd")
                        nc.scalar.activation(
                            out=sd[:], in_=mv[:, 1:2], func=AF.Sqrt,
                            bias=eps_t[:, :1],
                        )
                        rstd = stat.tile([P, 1], f32, tag="rstd", name="rstd")
                        nc.vector.reciprocal(out=rstd[:], in_=sd[:])
                        nmu = stat.tile([P, 1], f32, tag="nmu", name="nmu")
                        nc.vector.tensor_tensor(
                            out=nmu[:], in0=mv[:, 0:1], in1=rstd[:], op=ALU.mult
                        )
                        nc.vector.tensor_scalar_mul(
                            out=nmu[:], in0=nmu[:], scalar1=-1.0
                        )
                        core = corep.tile([P, D], bf16, tag="core", name="core")
                        nc.scalar.activation(
                            out=core[:],
                            in_=xs[:],
                            func=AF.Identity,
                            scale=rstd[:, :1],
                            bias=nmu[:, :1],
                        )
                        for k in range(KC):
                            tp = ptr.tile([P, P], bf16, tag="tp", name="tp")
                            nc.tensor.transpose(
                                out=tp[:],
                                in_=core[:, k * P : (k + 1) * P],
                                identity=ident[:],
                            )
                            nc.scalar.activation(
                                out=hN[k][:, b * P : (b + 1) * P],
                                in_=tp[:],
                                func=AF.Identity,
                                scale=gam_sb[:, ll, k : k + 1],
                                bias=bet_sb[:, ll, k : k + 1],
                            )

                    # ---- phase D: gate + blend (layers 1, 2)
                    if ll > 0:
                        for (t0, tw) in ftiles:
                            # compute ALL gate chunks for this tile before any
                            # blend writes hN (the matmuls read hN as input)
                            gs = []
                            for f in range(KC):
                                pg_ = pg.tile([P, 512], f32, tag="pg", name="pg")
                                for k in range(2 * KC):
                                    rhs_t = (hT if k < KC else hN)[k % KC]
                                    nc.tensor.matmul(
                                        pg_[:, :tw],
                                        lhsT=Wg_sb[:, k, f, :],
                                        rhs=rhs_t[:, t0 : t0 + tw],
                                        start=(k == 0),
                                        stop=(k == 2 * KC - 1),
                                    )
                                g = gtile.tile(
                                    [P, 512], bf16, tag=f"g{f}", name=f"g{f}"
                                )
                                nc.scalar.activation(
                                    out=g[:, :tw],
                                    in_=pg_[:, :tw],
                                    func=AF.Sigmoid,
                                    bias=bg_sb[:, f : f + 1],
                                )
                                gs.append(g)
                            for f in range(KC):
                                d_ = dtmp.tile(
                                    [P, 512], bf16, tag=f"d{f}", name=f"d{f}"
                                )
                                nc.vector.tensor_tensor(
                                    out=d_[:, :tw],
                                    in0=hN[f][:, t0 : t0 + tw],
                                    in1=hT[f][:, t0 : t0 + tw],
                                    op=ALU.subtract,
                                )
                                nc.vector.tensor_tensor(
                                    out=d_[:, :tw],
                                    in0=gs[f][:, :tw],
                                    in1=d_[:, :tw],
                                    op=ALU.mult,
                                )
                                nc.vector.tensor_tensor(
                                    out=hN[f][:, t0 : t0 + tw],
                                    in0=hT[f][:, t0 : t0 + tw],
                                    in1=d_[:, :tw],
                                    op=ALU.add,
                                )
                    hT, hN = hN, hT

                # ---- final: out = tanh(h) + rw * h_orig
                for b in range(NB):
                    ob = finp.tile([P, D], f32, tag="ob", name="ob")
                    for k in range(KC):
                        tp = ptr.tile([P, P], bf16, tag="tp", name="tp")
                        nc.tensor.transpose(
                            out=tp[:],
                            in_=hT[k][:, b * P : (b + 1) * P],
                            identity=ident[:],
                        )
                        nc.scalar.activation(
                            out=ob[:, k * P : (k + 1) * P], in_=tp[:],
                            func=AF.Tanh,
                        )
                    h0b = finp.tile([P, D], f32, tag="h0b", name="h0b")
                    nc.sync.dma_start(
                        out=h0b[:], in_=h0_d[b * P : (b + 1) * P, :]
                    )
                    rt = finp.tile([P, D], f32, tag="rt", name="rt")
                    nc.vector.tensor_scalar_mul(
                        out=rt[:], in0=h0b[:], scalar1=rw_sb[:, :1]
                    )
                    nc.vector.tensor_tensor(
                        out=ob[:], in0=ob[:], in1=rt[:], op=ALU.add
                    )
                    nc.sync.dma_start(
                        out=out_d[b * P : (b + 1) * P, :], in_=ob[:]
                    )

    _split_excess_waits(nc)
    return nc


_BUILD_CACHE = {}


def _get_nc(Kb):
    if Kb not in _BUILD_CACHE:
        _BUILD_CACHE[Kb] = _build(Kb)
    return _BUILD_CACHE[Kb]


def kernel(t=None, h=None, edge_index=None, W_gcn=None, b_gcn=None,
           ln_gamma=None, ln_beta=None, W_gate=None, b_gate=None,
           residual_weight=None, **_ignored):
    in_maps, Kb = _host_prep(
        h, edge_index, W_gcn, b_gcn, ln_gamma, ln_beta, W_gate, b_gate,
        residual_weight,
    )
    nc = _get_nc(Kb)
    res = run_bass_kernel_spmd(nc, in_maps, list(range(NC)))
    outs = [res.results[c]["out"][:SHARD] for c in range(NC)]
    return np.concatenate(outs, axis=0).astype(np.float32)


# revision 14
# speedup vs baseline: 39.5761x; 1.1866x over previous
"""Trainium2 Bass kernel for nn_NeuralODEFunc (3-layer gated GCN) on 8 NeuronCores.

Strategy (self-contained, hardcoded for N=50000, D=512, E=160000, 8 cores):
  - Nodes sharded across 8 cores (6250/core, padded to 6272 = 49 blocks of 128).
  - Edges partitioned by destination core/block; scatter-add is expressed as
    PE matmuls with per-block one-hot "S" matrices (norm baked in), built on host.
  - Per GCN layer: local XW matmul (bf16) -> AllGather (bf16) -> indirect-DMA
    gather of source rows -> S-matmul accumulate in PSUM -> bias + LayerNorm
    (bn_stats + ACT affine) -> PE transpose back to feature-major state ->
    gated blend (layers 1,2) via transposed gate matmuls + sigmoid.
  - Final tanh + residual_weight * h_orig, emitted row-major fp32.

State lives in SBUF as hT [4][128, 6272] bf16 (feature-major), double buffered.
"""

import time

import numpy as np
import ml_dtypes

import concourse.bass as bass
import concourse.mybir as mybir
import concourse.tile as tile
from concourse.bass_utils import run_bass_kernel_spmd
from concourse.masks import make_identity

NC = 8
P = 128
D = 512
N = 50000
E = 160000
SHARD = N // NC          # 6250
NB = (SHARD + P - 1) // P  # 49
SH = NB * P              # 6272
NPAD = NC * SH           # 50176
KC = D // P              # 4
LAYERS = 3
LN_EPS = 1e-5

bf16 = mybir.dt.bfloat16
f32 = mybir.dt.float32
i32 = mybir.dt.int32
AF = mybir.ActivationFunctionType
ALU = mybir.AluOpType


# ---------------------------------------------------------------- wait split
def _split_excess_waits(nc, max_waits=1):
    """This walrus build supports only ONE embedded sync wait per instruction.
    Move excess waits onto preceding same-engine NOPs (one wait each)."""
    n_split = 0
    for bb in nc.main_func.blocks:
        out = []
        changed = False
        for ins in bb.instructions:
            si = ins.sync_info
            if si is not None and len(si.on_wait) > max_waits:
                waits = list(si.on_wait)
                excess = waits[:-max_waits]
                keep = waits[-max_waits:]
                for w in excess:
                    nop = mybir.InstNoOp(
                        name=nc.get_next_instruction_name(),
                        text_hint="waitsplit",
                        bass_nofuse=True,
                    )
                    nop.engine = ins.engine
                    nop.sync_info = mybir.SyncInfo(on_wait=[w], on_update=[])
                    nc.register_instruction(nop)
                    out.append(nop)
                    n_split += 1
                ins.sync_info = mybir.SyncInfo(
                    on_wait=keep, on_update=list(si.on_update)
                )
                changed = True
            out.append(ins)
        if changed:
            bb.instructions = out
    return n_split


# ---------------------------------------------------------------- host prep
def _host_prep(h, edge_index, W_gcn, b_gcn, ln_gamma, ln_beta, W_gate, b_gate,
               residual_weight):
    src = np.asarray(edge_index[0], dtype=np.int64)
    dst = np.asarray(edge_index[1], dtype=np.int64)

    deg = np.bincount(dst, minlength=N).astype(np.float32)
    dinv = np.where(deg > 0, 1.0 / np.sqrt(np.maximum(deg, 1.0)), 0.0).astype(
        np.float32
    )
    norm = (dinv[src] * dinv[dst]).astype(np.float32)

    core_of = (dst // SHARD).astype(np.int64)
    loc = dst % SHARD
    blk = loc // P
    dloc = (loc % P).astype(np.int64)
    gsrc = ((src // SHARD) * SH + (src % SHARD)).astype(np.int32)

    # per (core, block) edge counts -> shared per-block chunk counts
    counts = np.zeros((NC, NB), np.int64)
    np.add.at(counts, (core_of, blk), 1)
    Kb = np.maximum(1, -(-counts.max(axis=0) // P)).astype(np.int64)  # ceil
    qofs = np.concatenate([[0], np.cumsum(Kb)]).astype(np.int64)
    Ksum = int(qofs[-1])

    # stable sort edges by (core, blk); position within group -> (chunk, lane)
    key = core_of * NB + blk
    order = np.argsort(key, kind="stable")
    skey = key[order]
    grp_start_mask = np.ones(E, dtype=bool)
    grp_start_mask[1:] = skey[1:] != skey[:-1]
    grp_start_idx = np.flatnonzero(grp_start_mask)
    grp_id = np.cumsum(grp_start_mask) - 1
    pos = np.arange(E) - grp_start_idx[grp_id]

    s_core = core_of[order]
    s_blk = blk[order]
    s_dloc = dloc[order]
    s_norm = norm[order]
    s_gsrc = gsrc[order]
    chunk = qofs[s_blk] + pos // P
    lane = pos % P

    S = np.zeros((NC, Ksum, P, P), np.float32)
    S[s_core, chunk, lane, s_dloc] = s_norm
    S = S.astype(ml_dtypes.bfloat16)
    idx = np.zeros((NC, Ksum, P), np.int32)
    idx[s_core, chunk, lane] = s_gsrc

    # padded node shards + transposed bf16 initial state
    h = np.asarray(h, dtype=np.float32)
    h_pad = np.zeros((NC, SH, D), np.float32)
    h_pad[:, :SHARD, :] = h.reshape(NC, SHARD, D)
    hT0 = (
        h_pad.transpose(0, 2, 1)  # [NC, D, SH]
        .reshape(NC, KC, P, SH)
        .transpose(0, 2, 1, 3)    # [NC, P, KC, SH]
        .astype(ml_dtypes.bfloat16)
    )

    Wg = np.asarray(W_gcn, np.float32).reshape(LAYERS, KC, P, D).astype(
        ml_dtypes.bfloat16
    )
    Wgate = np.asarray(W_gate, np.float32).reshape(2 * KC, P, KC, P).astype(
        ml_dtypes.bfloat16
    )
    gam = np.asarray(ln_gamma, np.float32).reshape(LAYERS, KC, P)
    bet = np.asarray(ln_beta, np.float32).reshape(LAYERS, KC, P)
    bgt = np.asarray(b_gate, np.float32).reshape(KC, P)
    bgc = np.asarray(b_gcn, np.float32)
    rw = np.asarray(residual_weight, np.float32).reshape(1, 1)

    trivial = (
        np.all(gam == 1.0) and np.all(bet == 0.0) and np.all(bgc == 0.0)
        and np.all(bgt == 0.0)
    )

    in_maps = []
    for c in range(NC):
        in_maps.append(
            {
                "hT0": hT0[c],
                "h0": h_pad[c],
                "S": S[c],
                "idx": idx[c],
                "Wgcn": Wg,
                "Wgate": Wgate,
                "bgcn": bgc,
                "gam": gam,
                "bet": bet,
                "bgate": bgt,
                "rw": rw,
            }
        )
    return in_maps, (tuple(int(k) for k in Kb), bool(trivial))


# ---------------------------------------------------------------- device build
def _build(Kb, trivial=True, repeats=1, skip_ag=False, skip_gather=False):
    qofs = [0]
    for k in Kb:
        qofs.append(qofs[-1] + k)
    Ksum = qofs[-1]

    nc = bass.Bass()

    hT0_d = nc.dram_tensor("hT0", [P, KC, SH], bf16, kind="ExternalInput")
    h0_d = nc.dram_tensor("h0", [SH, D], f32, kind="ExternalInput")
    S_d = nc.dram_tensor("S", [Ksum, P, P], bf16, kind="ExternalInput")
    idx_d = nc.dram_tensor("idx", [Ksum, P], i32, kind="ExternalInput")
    Wgcn_d = nc.dram_tensor("Wgcn", [LAYERS, KC, P, D], bf16, kind="ExternalInput")
    Wgate_d = nc.dram_tensor("Wgate", [2 * KC, P, KC, P], bf16, kind="ExternalInput")
    bgcn_d = nc.dram_tensor("bgcn", [LAYERS, D], f32, kind="ExternalInput")
    gam_d = nc.dram_tensor("gam", [LAYERS, KC, P], f32, kind="ExternalInput")
    bet_d = nc.dram_tensor("bet", [LAYERS, KC, P], f32, kind="ExternalInput")
    bgate_d = nc.dram_tensor("bgate", [KC, P], f32, kind="ExternalInput")
    rw_d = nc.dram_tensor("rw", [1, 1], f32, kind="ExternalInput")
    out_d = nc.dram_tensor("out", [SH, D], f32, kind="ExternalOutput")

    ag_in = [
        nc.dram_tensor(f"ag_in{l}", [SH, D], bf16)
        for l in range(LAYERS * repeats)
    ]
    ag_out = [
        nc.dram_tensor(f"ag_out{l}", [NPAD, D], bf16, addr_space="Shared")
        for l in range(LAYERS * repeats)
    ]

    # free tiles over SH for the gate phase
    ftiles = []
    o = 0
    while o < SH:
        w = min(512, SH - o)
        ftiles.append((o, w))
        o += w

    with tile.TileContext(nc) as tc:
        with (
            tc.tile_pool(name="const", bufs=1) as const,
            tc.tile_pool(name="state", bufs=1) as state,
            tc.tile_pool(name="xwp", bufs=3) as xwp,
            tc.tile_pool(name="msg", bufs=6) as msg,
            tc.tile_pool(name="spool", bufs=6) as spool,
            tc.tile_pool(name="xsp", bufs=3) as xsp,
            tc.tile_pool(name="stat", bufs=12) as stat,
            tc.tile_pool(name="corep", bufs=3) as corep,
            tc.tile_pool(name="gtile", bufs=3) as gtile,
            tc.tile_pool(name="dtmp", bufs=3) as dtmp,
            tc.tile_pool(name="finp", bufs=3) as finp,
            tc.tile_pool(name="pxw", bufs=2, space="PSUM") as pxw,
            tc.tile_pool(name="pagg", bufs=2, space="PSUM") as pagg,
            tc.tile_pool(name="ptr", bufs=2, space="PSUM") as ptr,
            tc.tile_pool(name="pg", bufs=2, space="PSUM") as pg,
        ):
            ident = const.tile([P, P], bf16)
            make_identity(nc, ident)
            eps_t = const.tile([P, 1], f32)
            nc.vector.memset(eps_t[:], LN_EPS)

            W_sb = const.tile([P, LAYERS, KC, D], bf16)
            nc.sync.dma_start(
                out=W_sb[:], in_=Wgcn_d.rearrange("l k p d -> p l k d")
            )
            Wg_sb = const.tile([P, 2 * KC, KC, P], bf16)
            nc.sync.dma_start(
                out=Wg_sb[:], in_=Wgate_d.rearrange("k p f c -> p k f c")
            )
            b_sb = const.tile([P, LAYERS, D], f32)
            nc.sync.dma_start(
                out=b_sb[:],
                in_=bass.AP(
                    tensor=bgcn_d[:].tensor,
                    offset=0,
                    ap=[[0, P]] + list(bgcn_d[:].ap),
                ),
            )
            gam_sb = const.tile([P, LAYERS, KC], f32)
            nc.sync.dma_start(out=gam_sb[:], in_=gam_d.rearrange("l k p -> p l k"))
            bet_sb = const.tile([P, LAYERS, KC], f32)
            nc.sync.dma_start(out=bet_sb[:], in_=bet_d.rearrange("l k p -> p l k"))
            bg_sb = const.tile([P, KC], f32)
            nc.sync.dma_start(out=bg_sb[:], in_=bgate_d.rearrange("f p -> p f"))
            rw_sb = const.tile([P, 1], f32)
            nc.sync.dma_start(
                out=rw_sb[:],
                in_=bass.AP(tensor=rw_d[:].tensor, offset=0, ap=[[0, P], [1, 1]]),
            )
            idx_sb = const.tile([P, Ksum], i32)
            nc.sync.dma_start(out=idx_sb[:], in_=idx_d.rearrange("q p -> p q"))
            probe = const.tile([P, 1], bf16)

            hT = state.tile([P, KC * SH], bf16, tag="hA", name="hA")
            hN = state.tile([P, KC * SH], bf16, tag="hB", name="hB")
            maxKb = max(Kb)

            # repeats > 1 is only for device-time measurement (slope method)
            for rep in range(repeats):
                nc.sync.dma_start(
                    out=hT[:], in_=hT0_d.rearrange("p k n -> p (k n)")
                )

                for ll in range(LAYERS):
                    lg = rep * LAYERS + ll  # ag buffer index
                    # ---- phase A: XW = h @ W_l (row-major bf16) -> ag_in
                    for b in range(NB):
                        ps = pxw.tile([P, D], f32, tag="pxw", name="pxw")
                        for k in range(KC):
                            nc.tensor.matmul(
                                ps[:],
                                lhsT=hT[:, k * SH + b * P : k * SH + (b + 1) * P],
                                rhs=W_sb[:, ll, k, :],
                                start=(k == 0),
                                stop=(k == KC - 1),
                            )
                        xw = xwp.tile([P, D], bf16, tag="xw", name="xw")
                        nc.scalar.activation(out=xw[:], in_=ps[:], func=AF.Copy)
                        nc.sync.dma_start(
                            out=ag_in[lg][b * P : (b + 1) * P, :], in_=xw[:]
                        )

                    # ---- AllGather full XW (bf16)
                    if not skip_ag:
                        nc.gpsimd.collective_compute(
                            "AllGather",
                            ALU.bypass,
                            ins=[ag_in[lg][:]],
                            outs=[ag_out[lg][:]],
                            replica_groups=[list(range(NC))],
                        )
                        # collapse the collective dep into the POOL queue
                        nc.gpsimd.dma_start(
                            out=probe[:1, :1], in_=ag_out[lg][0:1, 0:1]
                        )

                    # ---- phase B+C: gather, scatter-matmul, LN, transpose
                    for b in range(NB):
                        kb = Kb[b]
                        q0 = qofs[b]
                        s_t = spool.tile(
                            [P, maxKb * P], bf16, tag="s_t", name="s_t"
                        )
                        nc.sync.dma_start(
                            out=s_t[:, : kb * P].rearrange(
                                "p (q d) -> p q d", q=kb
                            ),
                            in_=S_d[q0 : q0 + kb].rearrange("q p d -> p q d"),
                        )
                        ps = pagg.tile([P, D], f32, tag="pagg", name="pagg")
                        for j in range(kb):
                            q = q0 + j
                            m = msg.tile([P, D], bf16, tag="m", name="m")
                            if not skip_gather:
                                nc.gpsimd.indirect_dma_start(
                                    out=m[:],
                                    out_offset=None,
                                    in_=ag_out[lg][:],
                                    in_offset=bass.IndirectOffsetOnAxis(
                                        ap=idx_sb[:, q : q + 1], axis=0
                                    ),
                                )
                            nc.tensor.matmul(
                                ps[:],
                                lhsT=s_t[:, j * P : (j + 1) * P],
                                rhs=m[:],
                                start=(j == 0),
                                stop=(j == kb - 1),
                            )
                        if trivial:
                            xstat = ps
                        else:
                            xs = xsp.tile([P, D], f32, tag="xs", name="xs")
                            nc.vector.tensor_tensor(
                                out=xs[:], in0=ps[:], in1=b_sb[:, ll, :],
                                op=ALU.add,
                            )
                            xstat = xs
                        st = stat.tile([P, 6], f32, tag="st", name="st")
                        nc.vector.bn_stats(out=st[:], in_=xstat[:])
                        mv = stat.tile([P, 2], f32, tag="mv", name="mv")
                        nc.vector.bn_aggr(out=mv[:], in_=st[:])
                        sd = stat.tile([P, 1], f32, tag="sd", name="sd")
                        nc.scalar.activation(
                            out=sd[:], in_=mv[:, 1:2], func=AF.Sqrt,
                            bias=eps_t[:, :1],
                        )
                        rstd = stat.tile([P, 1], f32, tag="rstd", name="rstd")
                        nc.vector.reciprocal(out=rstd[:], in_=sd[:])
                        nmu = stat.tile([P, 1], f32, tag="nmu", name="nmu")
                        nc.vector.tensor_tensor(
                            out=nmu[:], in0=mv[:, 0:1], in1=rstd[:], op=ALU.mult
                        )
                        nc.vector.tensor_scalar_mul(
                            out=nmu[:], in0=nmu[:], scalar1=-1.0
                        )
                        core = corep.tile([P, D], bf16, tag="core", name="core")
                        nc.scalar.activation(
                            out=core[:],
                            in_=xstat[:],
                            func=AF.Identity,
                            scale=rstd[:, :1],
                            bias=nmu[:, :1],
                        )
                        tpb = ptr.tile([P, KC, P], bf16, tag="tpb", name="tpb")
                        for k in range(KC):
                            nc.tensor.transpose(
                                out=tpb[:, k, :],
                                in_=core[:, k * P : (k + 1) * P],
                                identity=ident[:],
                            )
                        hN_dst = hN[:].rearrange("p (k n) -> p k n", k=KC)[
                            :, :, b * P : (b + 1) * P
                        ]
                        if trivial:
                            nc.scalar.activation(
                                out=hN_dst, in_=tpb[:], func=AF.Copy
                            )
                        else:
                            for k in range(KC):
                                nc.scalar.activation(
                                    out=hN[
                                        :,
                                        k * SH + b * P : k * SH + (b + 1) * P,
                                    ],
                                    in_=tpb[:, k, :],
                                    func=AF.Identity,
                                    scale=gam_sb[:, ll, k : k + 1],
                                    bias=bet_sb[:, ll, k : k + 1],
                                )

                    # ---- phase D: gate + blend (layers 1, 2)
                    if ll > 0:
                        for (t0, tw) in ftiles:
                            # compute ALL gate chunks for this tile before any
                            # blend writes hN (the matmuls read hN as input)
                            gs = []
                            for f in range(KC):
                                pg_ = pg.tile(
                                    [P, 512], f32, tag="pg", name="pg"
                                )
                                for k in range(2 * KC):
                                    rhs_t = hT if k < KC else hN
                                    kk = (k % KC) * SH
                                    nc.tensor.matmul(
                                        pg_[:, :tw],
                                        lhsT=Wg_sb[:, k, f, :],
                                        rhs=rhs_t[:, kk + t0 : kk + t0 + tw],
                                        start=(k == 0),
                                        stop=(k == 2 * KC - 1),
                                    )
                                g = gtile.tile(
                                    [P, 512], bf16, tag=f"g{f}", name=f"g{f}"
                                )
                                if trivial:
                                    nc.scalar.activation(
                                        out=g[:, :tw], in_=pg_[:, :tw],
                                        func=AF.Sigmoid,
                                    )
                                else:
                                    nc.scalar.activation(
                                        out=g[:, :tw],
                                        in_=pg_[:, :tw],
                                        func=AF.Sigmoid,
                                        bias=bg_sb[:, f : f + 1],
                                    )
                                gs.append(g)
                            for f in range(KC):
                                d_ = dtmp.tile(
                                    [P, 512], bf16, tag=f"d{f}", name=f"d{f}"
                                )
                                sl = slice(f * SH + t0, f * SH + t0 + tw)
                                nc.vector.tensor_tensor(
                                    out=d_[:, :tw],
                                    in0=hN[:, sl],
                                    in1=hT[:, sl],
                                    op=ALU.subtract,
                                )
                                nc.vector.tensor_tensor(
                                    out=d_[:, :tw],
                                    in0=gs[f][:, :tw],
                                    in1=d_[:, :tw],
                                    op=ALU.mult,
                                )
                                nc.vector.tensor_tensor(
                                    out=hN[:, sl],
                                    in0=hT[:, sl],
                                    in1=d_[:, :tw],
                                    op=ALU.add,
                                )
                    hT, hN = hN, hT

                # ---- final: out = tanh(h) + rw * h_orig
                for b in range(NB):
                    tpb = ptr.tile([P, KC, P], bf16, tag="tpb", name="tpb")
                    for k in range(KC):
                        nc.tensor.transpose(
                            out=tpb[:, k, :],
                            in_=hT[:, k * SH + b * P : k * SH + (b + 1) * P],
                            identity=ident[:],
                        )
                    ob = finp.tile([P, D], f32, tag="ob", name="ob")
                    nc.scalar.activation(out=ob[:], in_=tpb[:], func=AF.Tanh)
                    h0b = finp.tile([P, D], f32, tag="h0b", name="h0b")
                    nc.sync.dma_start(
                        out=h0b[:], in_=h0_d[b * P : (b + 1) * P, :]
                    )
                    rt = finp.tile([P, D], f32, tag="rt", name="rt")
                    nc.vector.tensor_scalar_mul(
                        out=rt[:], in0=h0b[:], scalar1=rw_sb[:, :1]
                    )
                    nc.vector.tensor_tensor(
                        out=ob[:], in0=ob[:], in1=rt[:], op=ALU.add
                    )
                    nc.sync.dma_start(
                        out=out_d[b * P : (b + 1) * P, :], in_=ob[:]
                    )

    _split_excess_waits(nc)
    return nc


_BUILD_CACHE = {}


def _get_nc(key):
    if key not in _BUILD_CACHE:
        Kb, trivial = key
        _BUILD_CACHE[key] = _build(Kb, trivial=trivial)
    return _BUILD_CACHE[key]


def kernel(t=None, h=None, edge_index=None, W_gcn=None, b_gcn=None,
           ln_gamma=None, ln_beta=None, W_gate=None, b_gate=None,
           residual_weight=None, **_ignored):
    in_maps, key = _host_prep(
        h, edge_index, W_gcn, b_gcn, ln_gamma, ln_beta, W_gate, b_gate,
        residual_weight,
    )
    nc = _get_nc(key)
    res = run_bass_kernel_spmd(nc, in_maps, list(range(NC)))
    outs = [res.results[c]["out"][:SHARD] for c in range(NC)]
    return np.concatenate(outs, axis=0).astype(np.float32)
